# revision 10
# baseline (speedup 1.0000x reference)
"""Multi-head attention (B=8, H=8, S=1024, d=128) on 8 TRN2 NeuronCores.

Strategy (v3)
-------------
- Job sharding: the 64 (batch, head) attention jobs are dealt so core c
  handles head c of ALL 8 batches. Every core then sees the same
  per-batch key-tile counts (compile shape = tuple(kt_b)), so keys are
  padded to each batch's own count (sum(kt_b) k-tiles per core) instead
  of the global max (8*max kt) -- ~10% less exp/matmul/DMA work.
- Host-side prep (layout only): per batch, compact keys/values to the
  seq_mask-selected rows (zero-padded per batch to kt_b 128-wide
  k-tiles), pre-transpose Q and K so the contraction dim lands on SBUF
  partitions, cast matmul operands to fp16. An indicator matrix
  ind[k, 32] rides along for the softmax denominator.
- All inputs are bulk-preloaded into SBUF (job-0 slices first on the
  HWDGE sync queue, the rest as 4 big DMAs on the gpsimd SWDGE queue);
  no per-job DMA stalls. A short burst of dummy matmuls warms the PE
  HAM clock-gate while the first DMAs land.
- Device math is a single software-pipelined stream over all k-tiles
  of all jobs. Cycle i runs concurrently on three engines:
    PE : QK  logitsT[k,q] = K^T[:, tile i+1].T @ Q^T   (fp16, 2 paired
         M=64 matmuls per q-half, diagonal PSUM banks)
    ACT: W^T = exp(logitsT * d^-0.5)  for tile i        (the pacer)
    PE : outT[d,q] += V.T @ W^T ; den[q] += ind.T @ W^T for tile i-1
  The one-cycle lag on the AV/den matmuls means every instruction's
  semaphores are long settled when it issues -- no engine round-trip
  stalls; ACT runs back-to-back at (1024+352)/1.2GHz per tile.
- Outputs leave as fp16 (numerator, and denominators packed 2 jobs per
  PSUM bank); the division happens on the host. The learned scalar
  bias b cancels in softmax. Fully-masked batches fall back to the
  uniform average on the host.
"""
from contextlib import ExitStack

import numpy as np

import concourse.bacc as bacc
import concourse.mybir as mybir
import concourse.tile as tile
from concourse.bass_utils import run_bass_kernel_spmd

F32 = mybir.dt.float32
F16 = mybir.dt.float16

B, S, D, H = 8, 1024, 1024, 8
DH = D // H              # 128, head dim = one partition tile
SCALE = float(DH) ** -0.5
NJ = 8                   # jobs per core (one per batch)

_NC_CACHE: dict[tuple, object] = {}

# build options (overridable for profiling experiments)
OPTS: dict = {}


def _build(kts: tuple, opts: dict | None = None):
    """Build + compile the per-core kernel; kts[j] = k-tiles of job j."""
    opts = opts or {}
    n_warm = opts.get("n_warm", 10)
    KT_TOT = sum(kts)
    K_TOT = KT_TOT * 128
    koff = [sum(kts[:j]) for j in range(NJ)]     # k-tile offset per job

    nc = bacc.Bacc("TRN2", target_bir_lowering=False, debug=False)

    q_t = nc.dram_tensor("q_t", [128, NJ * S], F16, kind="ExternalInput")
    k_t = nc.dram_tensor("k_t", [128, K_TOT], F16, kind="ExternalInput")
    v_c = nc.dram_tensor("v_c", [K_TOT, 128], F16, kind="ExternalInput")
    ind = nc.dram_tensor("ind", [K_TOT, 32], F16, kind="ExternalInput")
    out_t = nc.dram_tensor("out_t", [128, NJ * S], F16, kind="ExternalOutput")
    den_t = nc.dram_tensor("den_t", [NJ // 2, 128, 512], F16,
                           kind="ExternalOutput")

    with tile.TileContext(nc) as tc, ExitStack() as ctx:
        sb_k = ctx.enter_context(tc.tile_pool(name="sb_k", bufs=1))
        sb_q = ctx.enter_context(tc.tile_pool(name="sb_q", bufs=1))
        sb_v = ctx.enter_context(tc.tile_pool(name="sb_v", bufs=1))
        sb_ind = ctx.enter_context(tc.tile_pool(name="sb_ind", bufs=1))
        sb_wu = ctx.enter_context(tc.tile_pool(name="sb_wu", bufs=1))
        sb_w = ctx.enter_context(tc.tile_pool(name="sb_w", bufs=6))
        sb_out = ctx.enter_context(tc.tile_pool(name="sb_out", bufs=4))
        sb_den = ctx.enter_context(tc.tile_pool(name="sb_den", bufs=2))
        ps_l = ctx.enter_context(tc.tile_pool(name="ps_l", bufs=2, space="PSUM"))
        ps_o = ctx.enter_context(tc.tile_pool(name="ps_o", bufs=1, space="PSUM"))
        ps_d = ctx.enter_context(tc.tile_pool(name="ps_d", bufs=1, space="PSUM"))
        ps_w = ctx.enter_context(tc.tile_pool(name="ps_w", bufs=1, space="PSUM"))

        # ---- PE warm-up: dummy matmuls on a memset tile keep the PE busy
        # while the first input DMAs land, so HAM un-throttles early.
        wu = sb_wu.tile([128, 512], F16)
        nc.vector.memset(wu[:], 0)
        plw = ps_w.tile([64, 512], F32)
        for _ in range(n_warm):
            nc.tensor.matmul(plw[:], wu[:, 0:64], wu[:], start=True, stop=True,
                             skip_group_check=True)

        # ---- bulk input preload (job 0 slices first, on the sync HWDGE
        # queue; the big remainders on the gpsimd SWDGE queue) ----
        kth_all = sb_k.tile([128, K_TOT], F16)          # [d, k] per job
        qth_all = sb_q.tile([128, NJ * S], F16)         # [d, (j, q)]
        vh_all = sb_v.tile([128, KT_TOT * 128], F16)    # [k, (tile, d)]
        ind_sb = sb_ind.tile([128, KT_TOT * 32], F16)

        q3 = qth_all[:].rearrange("p (j q) -> p j q", j=NJ)
        v3 = vh_all[:].rearrange("p (t c) -> p t c", c=128)
        i3 = ind_sb[:].rearrange("p (t c) -> p t c", c=32)
        kp0 = kts[0] * 128
        nc.sync.dma_start(kth_all[:, 0:kp0], k_t.ap()[:, 0:kp0])
        nc.sync.dma_start(q3[:, 0:1, :], q_t.ap().rearrange(
            "p (j q) -> p j q", j=NJ)[:, 0:1, :])
        nc.sync.dma_start(
            v3[:, 0:kts[0], :],
            v_c.ap()[0:kp0, :].rearrange("(t p) c -> p t c", p=128))
        nc.sync.dma_start(
            i3[:, 0:kts[0], :],
            ind.ap()[0:kp0, :].rearrange("(t p) c -> p t c", p=128))
        nc.gpsimd.dma_start(kth_all[:, kp0:], k_t.ap()[:, kp0:])
        nc.gpsimd.dma_start(q3[:, 1:, :], q_t.ap().rearrange(
            "p (j q) -> p j q", j=NJ)[:, 1:, :])
        nc.gpsimd.dma_start(
            v3[:, kts[0]:, :],
            v_c.ap()[kp0:, :].rearrange("(t p) c -> p t c", p=128))
        nc.gpsimd.dma_start(
            i3[:, kts[0]:, :],
            ind.ap()[kp0:, :].rearrange("(t p) c -> p t c", p=128))

        s0, s1 = slice(0, 512), slice(512, 1024)

        # flat stream of (job, ktile) cycles
        cyc = [(j, kt) for j in range(NJ) for kt in range(kts[j])]
        N = len(cyc)
        pls, wts, po, pd = {}, {}, {}, {}

        def emit_qk(i):
            j, kt = cyc[i]
            pl = ps_l.tile([128, S], F32, tag="pl", name=f"pl_{i}")
            ks = (koff[j] + kt) * 128
            kA, kB = slice(ks, ks + 64), slice(ks + 64, ks + 128)
            qth = q3[:, j, :]
            nc.tensor.matmul(pl[0:64, s0], kth_all[:, kA], qth[:, s0])
            nc.tensor.matmul(pl[64:128, s1], kth_all[:, kB], qth[:, s1])
            nc.tensor.matmul(pl[64:128, s0], kth_all[:, kB], qth[:, s0])
            nc.tensor.matmul(pl[0:64, s1], kth_all[:, kA], qth[:, s1])
            pls[i] = pl

        def emit_exp(i):
            wt = sb_w.tile([128, S], F16, tag="wt", name=f"wt_{i}")
            nc.scalar.activation(
                wt[:], pls.pop(i)[:], mybir.ActivationFunctionType.Exp,
                scale=SCALE)
            wts[i] = wt

        def emit_avden(i):
            j, kt = cyc[i]
            first, last = kt == 0, kt == kts[j] - 1
            if first:
                po[j] = ps_o.tile([128, S], F32, tag="po", name=f"po_{j}")
                if j % 2 == 0:
                    pd[j // 2] = ps_d.tile([128, 512], F32, tag="pd",
                                           name=f"pd_{j // 2}")
            r0 = (j % 2) * 64
            pdj = pd[j // 2]
            poj = po[j]
            wt = wts.pop(i)
            t = koff[j] + kt
            ic = slice(t * 32, t * 32 + 32)
            vA, vB = v3[:, t, 0:64], v3[:, t, 64:128]
            seqs = [
                (pdj[r0:r0 + 32, :], ind_sb[:, ic], wt[:, s0], (0, r0)),
                (pdj[r0 + 32:r0 + 64, :], ind_sb[:, ic], wt[:, s1],
                 (0, r0 + 32)),
                (poj[0:64, s0], vA, wt[:, s0], None),
                (poj[64:128, s1], vB, wt[:, s1], None),
                (poj[64:128, s0], vB, wt[:, s0], None),
                (poj[0:64, s1], vA, wt[:, s1], None),
            ]
            for out_ap, w_ap, r_ap, tp in seqs:
                nc.tensor.matmul(out_ap, w_ap, r_ap, start=first, stop=last,
                                 tile_position=tp)
            if last:
                # split copies so the s0 half of po frees up sooner
                osb = sb_out.tile([128, S], F16, tag="osb", name=f"osb_{j}")
                poj = po.pop(j)
                nc.vector.tensor_copy(osb[:, s0], poj[:, s0])
                nc.vector.tensor_copy(osb[:, s1], poj[:, s1])
                nc.sync.dma_start(
                    out_t.ap()[:, j * S:(j + 1) * S], osb[:])
                if j % 2 == 1:
                    dsb = sb_den.tile([128, 512], F16, tag="dsb",
                                      name=f"dsb_{j // 2}")
                    nc.vector.tensor_copy(dsb[:], pd.pop(j // 2)[:])
                    nc.sync.dma_start(den_t.ap()[j // 2, :, :], dsb[:])

        # software-pipelined stream: cycle i = QK(i+1) | exp(i) | AVden(i-1)
        emit_qk(0)
        for i in range(N):
            if i + 1 < N:
                emit_qk(i + 1)
            emit_exp(i)
            if i > 0:
                emit_avden(i - 1)
        emit_avden(N - 1)

    nc.compile()
    return nc


def kernel(memory, query, seq_mask, b):
    memory = np.ascontiguousarray(memory, dtype=np.float32)
    query = np.ascontiguousarray(query, dtype=np.float32)
    seq_mask = np.asarray(seq_mask)
    assert memory.shape == (B, S, 2 * D) and query.shape == (B, S, D)

    counts = [int(np.count_nonzero(seq_mask[i])) for i in range(B)]
    kps = [max(((c + 127) // 128) * 128, 128) for c in counts]
    kts = tuple(kp // 128 for kp in kps)
    K_TOT = sum(kps)

    key = (kts, tuple(sorted(OPTS.items())))
    if key not in _NC_CACHE:
        _NC_CACHE[key] = _build(kts, OPTS)
    nc = _NC_CACHE[key]

    # shared compacted K/V/ind, stacked per batch along k
    k_allT = np.zeros((D, K_TOT), dtype=np.float16)
    v_all = np.zeros((K_TOT, D), dtype=np.float16)
    ind_all = np.zeros((K_TOT, 32), dtype=np.float16)
    off = 0
    offs = []
    for i in range(B):
        idx = np.flatnonzero(seq_mask[i])
        nb = len(idx)
        offs.append(off)
        if nb:
            k_allT[:, off:off + nb] = memory[i, idx, :D].T
            v_all[off:off + nb] = memory[i, idx, D:]
            ind_all[off:off + nb] = 1.0
        off += kps[i]

    q_t = query.transpose(0, 2, 1).astype(np.float16)     # [B, D, S]
    in_maps = []
    for c in range(B):
        hs = c * DH
        qc = np.ascontiguousarray(
            q_t[:, hs:hs + DH, :].transpose(1, 0, 2).reshape(DH, B * S))
        kc = np.ascontiguousarray(k_allT[hs:hs + DH])
        vc = np.ascontiguousarray(v_all[:, hs:hs + DH])
        in_maps.append({"q_t": qc, "k_t": kc, "v_c": vc, "ind": ind_all})

    res = run_bass_kernel_spmd(nc, in_maps, list(range(B)))
    out = np.empty((B, S, D), dtype=np.float32)
    for c in range(B):
        hs = c * DH
        num = res.results[c]["out_t"].astype(np.float32)   # [DH, B*S]
        dd = res.results[c]["den_t"].astype(np.float32)    # [4, 128, 512]
        for j in range(B):
            blk = dd[j // 2]
            r0 = (j % 2) * 64
            den = np.concatenate([blk[r0], blk[r0 + 32]])  # [S]
            with np.errstate(divide="ignore", invalid="ignore"):
                out[j, :, hs:hs + DH] = (num[:, j * S:(j + 1) * S] /
                                         den[None, :]).T
    for i in range(B):
        if counts[i] == 0:
            out[i] = memory[i, :, D:].mean(axis=0)[None, :]
    return out
